# revision 1
# baseline (speedup 1.0000x reference)
"""Local (windowed) attention with RoPE for Trainium2, SPMD over 8 NeuronCores.

Reference semantics (nn_LocalAttention): B,H,N,D = 4,16,4096,64, window=128,
look_backward=1, look_forward=0, pad_value=-1 (pad applies to k/v VALUES and
to the position ids; padded keys end up unmasked all -1.0 vectors).

Sharding: merged (B*H)=64 leading dim split across 8 cores, 8 slices each.
Everything else runs per-core with no collectives.
"""

import numpy as np
import ml_dtypes

import concourse.bass as bass
import concourse.bacc as bacc
import concourse.mybir as mybir
import concourse.tile as tile
from concourse.bass_utils import run_bass_kernel_spmd

F32 = mybir.dt.float32
BF16 = mybir.dt.bfloat16
NP_BF16 = ml_dtypes.bfloat16

B, H, N, D = 4, 16, 4096, 64
W = 128                    # window size
NCORES = 8
BH = B * H
BH_PER_CORE = BH // NCORES
SCALE = float(D) ** -0.5
HD = D // 2


def rope_tables(n):
    """cos/sin tables matching the reference's fp32 computation.

    sinm folds the rotate_half sign: q'[d] = q[d]*cos[d] + q[(d+32)%64]*sinm[d].
    """
    inv_freq = 1.0 / (10000.0 ** (np.arange(0, D, 2, dtype=np.float32) / np.float32(D)))
    t = np.arange(n, dtype=np.float32)
    half = t[:, None] * inv_freq[None, :]
    freqs = np.concatenate([half, half], axis=-1)  # [n, D]
    cos = np.cos(freqs).astype(np.float32)
    sin = np.sin(freqs).astype(np.float32)
    sinm = np.concatenate([-sin[:, :HD], sin[:, HD:]], axis=-1)
    return cos, sinm


def host_consts(n):
    cos, sinm = rope_tables(n)
    # tri[j, i] = 1 where key j <= query i (window-local causal keep-mask)
    j = np.arange(W)[:, None]
    i = np.arange(W)[None, :]
    tri = (j <= i).astype(NP_BF16)
    ident = np.eye(D + 1, dtype=np.float32)
    return {
        "cos_t": cos.astype(NP_BF16),
        "sinm_t": sinm.astype(NP_BF16),
        "tri": tri,
        "id65": ident,
    }


def build_nc(bh_per_core=BH_PER_CORE, n=N):
    nw = n // W
    assert nw % 2 == 0
    ns = nw // 2  # transpose slabs (2 windows each)

    nc = bacc.Bacc(None, target_bir_lowering=False)
    q_d = nc.dram_tensor("q", [bh_per_core, n, D], F32, kind="ExternalInput")
    k_d = nc.dram_tensor("k", [bh_per_core, n, D], F32, kind="ExternalInput")
    v_d = nc.dram_tensor("v", [bh_per_core, n, D], F32, kind="ExternalInput")
    cos_d = nc.dram_tensor("cos_t", [n, D], BF16, kind="ExternalInput")
    sinm_d = nc.dram_tensor("sinm_t", [n, D], BF16, kind="ExternalInput")
    tri_d = nc.dram_tensor("tri", [W, W], BF16, kind="ExternalInput")
    id_d = nc.dram_tensor("id65", [D + 1, D + 1], F32, kind="ExternalInput")
    o_d = nc.dram_tensor("out", [bh_per_core, n, D], F32, kind="ExternalOutput")

    def nat(ap):  # DRAM [n, D] -> [t, w, d] token-in-window on partitions
        return ap.rearrange("(w t) d -> t w d", t=W)

    with tile.TileContext(nc) as tc:
        with (
            tc.tile_pool(name="const", bufs=1) as constp,
            tc.tile_pool(name="io", bufs=2) as iop,
            tc.tile_pool(name="rope", bufs=2) as ropep,
            tc.tile_pool(name="stk", bufs=2) as stkp,
            tc.tile_pool(name="esb", bufs=4) as ep,
            tc.tile_pool(name="otsb", bufs=6) as otp,
            tc.tile_pool(name="rsb", bufs=3) as rp,
            tc.tile_pool(name="stage", bufs=2) as stagep,
            tc.tile_pool(name="psim", bufs=2, space="PSUM") as psimp,
            tc.tile_pool(name="pS", bufs=4, space="PSUM") as pSp,
            tc.tile_pool(name="pO", bufs=2, space="PSUM") as pOp,
        ):
            cos_sb = constp.tile([W, nw, D], BF16, tag="cos")
            nc.sync.dma_start(out=cos_sb, in_=nat(cos_d))
            sinm_sb = constp.tile([W, nw, D], BF16, tag="sinm")
            nc.sync.dma_start(out=sinm_sb, in_=nat(sinm_d))
            tri_sb = constp.tile([W, W], BF16, tag="tri")
            nc.sync.dma_start(out=tri_sb, in_=tri_d[:])
            id_sb = constp.tile([D + 1, D + 1], F32, tag="id65")
            nc.sync.dma_start(out=id_sb, in_=id_d[:])
            kpadT = constp.tile([D, W], BF16, tag="kpadT")
            nc.vector.memset(kpadT[:], -1.0)
            vpad = constp.tile([W, D + 1], BF16, tag="vpad")
            nc.vector.memset(vpad[:], -1.0)
            nc.vector.memset(vpad[:, D : D + 1], 1.0)

            for bh in range(bh_per_core):
                qn = iop.tile([W, nw, D], F32, tag="qn")
                nc.sync.dma_start(out=qn[:], in_=nat(q_d[bh]))
                kn = iop.tile([W, nw, D], F32, tag="kn")
                nc.sync.dma_start(out=kn[:], in_=nat(k_d[bh]))
                vn = iop.tile([W, nw, D], F32, tag="vn")
                nc.sync.dma_start(out=vn[:], in_=nat(v_d[bh]))

                # ---- RoPE (bf16, natural layout) ----
                # Output tiles are [W, nw, 2D] with d-columns D:2D zero -- the
                # XBAR transpose then puts every window's d-major tile at
                # partitions 0:64 (uniform matmul base partition).
                def rope(xn, tag):
                    xb = ropep.tile([W, nw, D], BF16, tag=tag + "b")
                    nc.vector.tensor_copy(out=xb[:], in_=xn[:])
                    xr = ropep.tile([W, nw, D], BF16, tag=tag + "r")
                    nc.vector.tensor_mul(
                        out=xr[:, :, 0:HD], in0=xb[:, :, HD:D], in1=sinm_sb[:, :, 0:HD]
                    )
                    nc.vector.tensor_mul(
                        out=xr[:, :, HD:D], in0=xb[:, :, 0:HD], in1=sinm_sb[:, :, HD:D]
                    )
                    xp = ropep.tile([W, nw, 2 * D], BF16, tag=tag + "p")
                    if bh < 2:  # zero the pad lanes once per pool slot
                        nc.vector.memset(xp[:, :, D : 2 * D], 0.0)
                    nc.vector.tensor_mul(out=xp[:, :, 0:D], in0=xb[:], in1=cos_sb[:])
                    nc.vector.tensor_add(
                        out=xp[:, :, 0:D], in0=xp[:, :, 0:D], in1=xr[:]
                    )
                    return xp

                qp = rope(qn, "q")
                kp = rope(kn, "k")

                # v in bf16 with a fused ones column (denominator row of S)
                vb = ropep.tile([W, nw, D + 1], BF16, tag="vb")
                nc.vector.memset(vb[:, :, D : D + 1], 1.0)
                nc.scalar.copy(out=vb[:, :, 0:D], in_=vn[:])

                # ---- d-major via XBAR dma transpose ----
                # stq[p, w, t]: p<64 -> d of window w; p>=64 -> zero pad
                stq = stkp.tile([W, nw, W], BF16, tag="stq")
                nc.sync.dma_start(
                    out=stq[:], in_=qp.rearrange("t w d -> t (w d)"), transpose=True
                )
                stk = stkp.tile([W, nw, W], BF16, tag="stk")
                nc.sync.dma_start(
                    out=stk[:], in_=kp.rearrange("t w d -> t (w d)"), transpose=True
                )

                def qT(w):  # [64, 128] moving operand for queries of window w
                    return stq[0:D, w, :]

                def kT(w):  # [64, 128] stationary operand for keys of window w
                    return stk[0:D, w, :]

                # groups of key blocks: g=0 -> (pad, 0); 1..ns-1 -> (2g-1, 2g);
                # g=ns -> (nw-1,)
                e_tiles = {}  # c -> (E tile, slot)
                o_quads = {}
                stage_sb = stagep.tile([W, nw, D], F32, tag="stage")

                def do_window(w):
                    # out^T (and denom) for window w: accumulate both key
                    # blocks' PV into one PSUM tile, evacuate, transpose.
                    et0, sl0 = e_tiles[w - 1]
                    et1, sl1 = e_tiles[w]
                    pw = pSp.tile([D + 1, W], F32, tag="s", name="pw")
                    if w == 0:
                        nc.tensor.matmul(
                            pw[:], vpad[:], et0[:, sl0, 0:W], start=True, stop=False
                        )
                    else:
                        nc.tensor.matmul(
                            pw[:], vb[:, w - 1, :], et0[:, sl0, W : 2 * W],
                            start=True, stop=False,
                        )
                    nc.tensor.matmul(
                        pw[:], vb[:, w, :], et1[:, sl1, 0:W], start=False, stop=True
                    )
                    ot = otp.tile([D + 1, W], F32, tag="ot")
                    if w % 4 == 2:  # shed some PSUM-evac load from DVE to ACT
                        nc.scalar.copy(out=ot[:], in_=pw[:])
                    else:
                        nc.vector.tensor_copy(out=ot[:], in_=pw[:])
                    qi = w // 4
                    if qi not in o_quads:
                        o_quads[qi] = pOp.tile([W, 4, D + 1], F32, tag="oq", name="oq")
                    oq = o_quads[qi]
                    sl = w % 4
                    nc.tensor.transpose(oq[:, sl, :], ot[:], id_sb[:])
                    if sl == 3 or w == nw - 1:
                        nsl = sl + 1
                        r = rp.tile([W, 4], F32, tag="r")
                        nc.vector.reciprocal(
                            out=r[:, 0:nsl], in_=oq[:, 0:nsl, D : D + 1]
                        )
                        for j in range(nsl):
                            ww = qi * 4 + j
                            nc.scalar.activation(
                                out=stage_sb[:, ww, :],
                                in_=oq[:, j, 0:D],
                                func=mybir.ActivationFunctionType.Copy,
                                scale=r[:, j : j + 1],
                            )

                for g in range(ns + 1):
                    blocks = (
                        [-1, 0] if g == 0 else ([nw - 1] if g == ns else [2 * g - 1, 2 * g])
                    )
                    simt = psimp.tile([W, 2, 2 * W], F32, tag="sim")
                    et = ep.tile([W, 2, 2 * W], BF16, tag="e")
                    for sl, c in enumerate(blocks):
                        last = c == nw - 1
                        if c == -1:
                            nc.tensor.matmul(
                                simt[:, sl, 0:W], kpadT[:], qT(0), start=True, stop=True
                            )
                        else:
                            nc.tensor.matmul(
                                simt[:, sl, 0:W], kT(c), qT(c), start=True, stop=True
                            )
                            if not last:
                                nc.tensor.matmul(
                                    simt[:, sl, W : 2 * W],
                                    kT(c),
                                    qT(c + 1),
                                    start=True,
                                    stop=True,
                                )
                    # exp (scale folded); masked entries fixed up after
                    if g == 0:
                        nc.scalar.activation(
                            out=et[:, 0, 0:W], in_=simt[:, 0, 0:W],
                            func=mybir.ActivationFunctionType.Exp, scale=SCALE,
                        )
                        nc.scalar.activation(
                            out=et[:, 1, :], in_=simt[:, 1, :],
                            func=mybir.ActivationFunctionType.Exp, scale=SCALE,
                        )
                        nc.vector.tensor_mul(
                            out=et[:, 1, 0:W], in0=et[:, 1, 0:W], in1=tri_sb[:]
                        )
                    elif g == ns:
                        nc.scalar.activation(
                            out=et[:, 0, 0:W], in_=simt[:, 0, 0:W],
                            func=mybir.ActivationFunctionType.Exp, scale=SCALE,
                        )
                        nc.vector.tensor_mul(
                            out=et[:, 0, 0:W], in0=et[:, 0, 0:W], in1=tri_sb[:]
                        )
                    else:
                        nc.scalar.activation(
                            out=et[:, :, :], in_=simt[:, :, :],
                            func=mybir.ActivationFunctionType.Exp, scale=SCALE,
                        )
                        for sl in range(2):
                            nc.vector.tensor_mul(
                                out=et[:, sl, 0:W], in0=et[:, sl, 0:W], in1=tri_sb[:]
                            )
                    for sl, c in enumerate(blocks):
                        e_tiles[c] = (et, sl)
                    # windows ready after this group
                    for w in ([0] if g == 0 else ([nw - 1] if g == ns else [2 * g - 1, 2 * g])):
                        do_window(w)
                        e_tiles.pop(w - 1, None)

                nc.sync.dma_start(out=nat(o_d[bh]), in_=stage_sb[:])

    nc.finalize()
    return nc


_built = {}
TRACE = False
LAST_RESULT = None


def _get_nc(bh_per_core=BH_PER_CORE, n=N):
    key = (bh_per_core, n)
    if key not in _built:
        _built[key] = build_nc(bh_per_core, n)
    return _built[key]


def kernel(q, k, v):
    assert q.shape == (B, H, N, D)
    qf = np.ascontiguousarray(q, dtype=np.float32).reshape(BH, N, D)
    kf = np.ascontiguousarray(k, dtype=np.float32).reshape(BH, N, D)
    vf = np.ascontiguousarray(v, dtype=np.float32).reshape(BH, N, D)
    consts = host_consts(N)
    nc = _get_nc()
    in_maps = []
    for c in range(NCORES):
        s = slice(c * BH_PER_CORE, (c + 1) * BH_PER_CORE)
        in_maps.append({"q": qf[s], "k": kf[s], "v": vf[s], **consts})
    global LAST_RESULT
    res = run_bass_kernel_spmd(nc, in_maps, list(range(NCORES)), trace=TRACE)
    LAST_RESULT = res
    out = np.concatenate([res.results[i]["out"] for i in range(NCORES)], axis=0)
    return out.reshape(B, H, N, D).astype(np.float32)



# revision 4
# speedup vs baseline: 2.1824x; 2.1824x over previous
"""Local (windowed) attention with RoPE for Trainium2, SPMD over 8 NeuronCores.

Reference semantics (nn_LocalAttention): B,H,N,D = 4,16,4096,64, window=128,
look_backward=1, look_forward=0, pad_value=-1 (pad applies to k/v VALUES and
to the position ids; padded keys end up unmasked all -1.0 vectors).

Sharding: merged (B*H)=64 leading dim split across 8 cores, 8 slices each.
Everything else runs per-core with no collectives.
"""

import numpy as np
import ml_dtypes

import concourse.bass as bass
import concourse.bacc as bacc
import concourse.mybir as mybir
import concourse.tile as tile
from concourse.bass_utils import run_bass_kernel_spmd

F32 = mybir.dt.float32
BF16 = mybir.dt.bfloat16
NP_BF16 = ml_dtypes.bfloat16

B, H, N, D = 4, 16, 4096, 64
W = 128                    # window size
NCORES = 8
BH = B * H
BH_PER_CORE = BH // NCORES
SCALE = float(D) ** -0.5
HD = D // 2


def rope_tables(n):
    """cos/sin tables matching the reference's fp32 computation.

    sinm folds the rotate_half sign: q'[d] = q[d]*cos[d] + q[(d+32)%64]*sinm[d].
    """
    inv_freq = 1.0 / (10000.0 ** (np.arange(0, D, 2, dtype=np.float32) / np.float32(D)))
    t = np.arange(n, dtype=np.float32)
    half = t[:, None] * inv_freq[None, :]
    freqs = np.concatenate([half, half], axis=-1)  # [n, D]
    cos = np.cos(freqs).astype(np.float32)
    sin = np.sin(freqs).astype(np.float32)
    sinm = np.concatenate([-sin[:, :HD], sin[:, HD:]], axis=-1)
    return cos, sinm


def host_consts(n):
    cos, sinm = rope_tables(n)
    # tri[j, i] = 1 where key j <= query i (window-local causal keep-mask)
    j = np.arange(W)[:, None]
    i = np.arange(W)[None, :]
    tri = (j <= i).astype(NP_BF16)
    ident = np.eye(D + 1, dtype=np.float32)
    return {
        "cos_t": cos.astype(NP_BF16),
        "sinm_t": sinm.astype(NP_BF16),
        "tri": tri,
        "id65": ident,
    }


def build_nc(bh_per_core=BH_PER_CORE, n=N):
    nw = n // W
    assert nw % 2 == 0
    ns = nw // 2  # transpose slabs (2 windows each)

    nc = bacc.Bacc(None, target_bir_lowering=False)
    q_d = nc.dram_tensor("q", [bh_per_core, n, D], F32, kind="ExternalInput")
    k_d = nc.dram_tensor("k", [bh_per_core, n, D], F32, kind="ExternalInput")
    v_d = nc.dram_tensor("v", [bh_per_core, n, D], F32, kind="ExternalInput")
    cos_d = nc.dram_tensor("cos_t", [n, D], BF16, kind="ExternalInput")
    sinm_d = nc.dram_tensor("sinm_t", [n, D], BF16, kind="ExternalInput")
    tri_d = nc.dram_tensor("tri", [W, W], BF16, kind="ExternalInput")
    id_d = nc.dram_tensor("id65", [D + 1, D + 1], F32, kind="ExternalInput")
    o_d = nc.dram_tensor("out", [bh_per_core, n, D], F32, kind="ExternalOutput")

    def nat(ap):  # DRAM [n, D] -> [t, w, d] token-in-window on partitions
        return ap.rearrange("(w t) d -> t w d", t=W)

    with tile.TileContext(nc) as tc:
        with (
            tc.tile_pool(name="const", bufs=1) as constp,
            tc.tile_pool(name="io", bufs=2) as iop,
            tc.tile_pool(name="rope", bufs=2) as ropep,
            tc.tile_pool(name="stk", bufs=2) as stkp,
            tc.tile_pool(name="esb", bufs=4) as ep,
            tc.tile_pool(name="otsb", bufs=6) as otp,
            tc.tile_pool(name="rsb", bufs=3) as rp,
            tc.tile_pool(name="stage", bufs=2) as stagep,
            tc.tile_pool(name="psim", bufs=2, space="PSUM") as psimp,
            tc.tile_pool(name="pS", bufs=4, space="PSUM") as pSp,
            tc.tile_pool(name="pO", bufs=2, space="PSUM") as pOp,
        ):
            cos_sb = constp.tile([W, nw, D], BF16, tag="cos")
            nc.sync.dma_start(out=cos_sb, in_=nat(cos_d))
            sinm_sb = constp.tile([W, nw, D], BF16, tag="sinm")
            nc.sync.dma_start(out=sinm_sb, in_=nat(sinm_d))
            tri_sb = constp.tile([W, W], BF16, tag="tri")
            nc.sync.dma_start(out=tri_sb, in_=tri_d[:])
            id_sb = constp.tile([D + 1, D + 1], F32, tag="id65")
            nc.sync.dma_start(out=id_sb, in_=id_d[:])
            kpadT = constp.tile([D, W], BF16, tag="kpadT")
            nc.vector.memset(kpadT[:], -1.0)
            vpad = constp.tile([W, D + 1], BF16, tag="vpad")
            nc.vector.memset(vpad[:], -1.0)
            nc.vector.memset(vpad[:, D : D + 1], 1.0)

            for bh in range(bh_per_core):
                qn = iop.tile([W, nw, D], F32, tag="qn")
                nc.sync.dma_start(out=qn[:], in_=nat(q_d[bh]))
                kn = iop.tile([W, nw, D], F32, tag="kn")
                nc.sync.dma_start(out=kn[:], in_=nat(k_d[bh]))
                vn = iop.tile([W, nw, D], F32, tag="vn")
                nc.sync.dma_start(out=vn[:], in_=nat(v_d[bh]))

                # ---- RoPE (bf16, natural layout) ----
                # Output tiles are [W, nw, 2D] with d-columns D:2D zero -- the
                # XBAR transpose then puts every window's d-major tile at
                # partitions 0:64 (uniform matmul base partition).
                def rope(xn, tag):
                    xb = ropep.tile([W, nw, D], BF16, tag=tag + "b")
                    nc.vector.tensor_copy(out=xb[:], in_=xn[:])
                    xr = ropep.tile([W, nw, D], BF16, tag=tag + "r")
                    nc.vector.tensor_mul(
                        out=xr[:, :, 0:HD], in0=xb[:, :, HD:D], in1=sinm_sb[:, :, 0:HD]
                    )
                    nc.vector.tensor_mul(
                        out=xr[:, :, HD:D], in0=xb[:, :, 0:HD], in1=sinm_sb[:, :, HD:D]
                    )
                    xp = ropep.tile([W, nw, 2 * D], BF16, tag=tag + "p")
                    if bh < 2:  # zero the pad lanes once per pool slot
                        nc.vector.memset(xp[:, :, D : 2 * D], 0.0)
                    nc.vector.tensor_mul(out=xp[:, :, 0:D], in0=xb[:], in1=cos_sb[:])
                    nc.vector.tensor_add(
                        out=xp[:, :, 0:D], in0=xp[:, :, 0:D], in1=xr[:]
                    )
                    return xp

                qp = rope(qn, "q")
                kp = rope(kn, "k")

                # v in bf16 with a fused ones column (denominator row of S)
                vb = ropep.tile([W, nw, D + 1], BF16, tag="vb")
                nc.vector.memset(vb[:, :, D : D + 1], 1.0)
                nc.scalar.copy(out=vb[:, :, 0:D], in_=vn[:])

                # ---- d-major via XBAR dma transpose ----
                # stq[p, w, t]: p<64 -> d of window w; p>=64 -> zero pad
                stq = stkp.tile([W, nw, W], BF16, tag="stq")
                nc.sync.dma_start(
                    out=stq[:], in_=qp.rearrange("t w d -> t (w d)"), transpose=True
                )
                stk = stkp.tile([W, nw, W], BF16, tag="stk")
                nc.sync.dma_start(
                    out=stk[:], in_=kp.rearrange("t w d -> t (w d)"), transpose=True
                )

                def qT(w):  # [64, 128] moving operand for queries of window w
                    return stq[0:D, w, :]

                def kT(w):  # [64, 128] stationary operand for keys of window w
                    return stk[0:D, w, :]

                # groups of key blocks: g=0 -> (pad, 0); 1..ns-1 -> (2g-1, 2g);
                # g=ns -> (nw-1,)
                e_tiles = {}  # c -> (E tile, slot)
                o_quads = {}
                stage_sb = stagep.tile([W, nw, D], F32, tag="stage")

                def do_window(w):
                    # out^T (and denom) for window w: accumulate both key
                    # blocks' PV into one PSUM tile, evacuate, transpose.
                    et0, sl0 = e_tiles[w - 1]
                    et1, sl1 = e_tiles[w]
                    pw = pSp.tile([D + 1, W], F32, tag="s", name="pw")
                    if w == 0:
                        nc.tensor.matmul(
                            pw[:], vpad[:], et0[:, sl0, 0:W], start=True, stop=False
                        )
                    else:
                        nc.tensor.matmul(
                            pw[:], vb[:, w - 1, :], et0[:, sl0, W : 2 * W],
                            start=True, stop=False,
                        )
                    nc.tensor.matmul(
                        pw[:], vb[:, w, :], et1[:, sl1, 0:W], start=False, stop=True
                    )
                    ot = otp.tile([D + 1, W], F32, tag="ot")
                    if w % 4 == 2:  # shed some PSUM-evac load from DVE to ACT
                        nc.scalar.copy(out=ot[:], in_=pw[:])
                    else:
                        nc.vector.tensor_copy(out=ot[:], in_=pw[:])
                    qi = w // 4
                    if qi not in o_quads:
                        o_quads[qi] = pOp.tile([W, 4, D + 1], F32, tag="oq", name="oq")
                    oq = o_quads[qi]
                    sl = w % 4
                    nc.tensor.transpose(oq[:, sl, :], ot[:], id_sb[:])
                    if sl == 3 or w == nw - 1:
                        nsl = sl + 1
                        r = rp.tile([W, 4], F32, tag="r")
                        nc.vector.reciprocal(
                            out=r[:, 0:nsl], in_=oq[:, 0:nsl, D : D + 1]
                        )
                        for j in range(nsl):
                            ww = qi * 4 + j
                            nc.scalar.activation(
                                out=stage_sb[:, ww, :],
                                in_=oq[:, j, 0:D],
                                func=mybir.ActivationFunctionType.Copy,
                                scale=r[:, j : j + 1],
                            )

                for g in range(ns + 1):
                    blocks = (
                        [-1, 0] if g == 0 else ([nw - 1] if g == ns else [2 * g - 1, 2 * g])
                    )
                    simt = psimp.tile([W, 2, 2 * W], F32, tag="sim")
                    et = ep.tile([W, 2, 2 * W], BF16, tag="e")
                    for sl, c in enumerate(blocks):
                        last = c == nw - 1
                        if c == -1:
                            nc.tensor.matmul(
                                simt[:, sl, 0:W], kpadT[:], qT(0), start=True, stop=True
                            )
                        else:
                            nc.tensor.matmul(
                                simt[:, sl, 0:W], kT(c), qT(c), start=True, stop=True
                            )
                            if not last:
                                nc.tensor.matmul(
                                    simt[:, sl, W : 2 * W],
                                    kT(c),
                                    qT(c + 1),
                                    start=True,
                                    stop=True,
                                )
                    # exp (scale folded); masked entries fixed up after
                    if g == 0:
                        nc.scalar.activation(
                            out=et[:, 0, 0:W], in_=simt[:, 0, 0:W],
                            func=mybir.ActivationFunctionType.Exp, scale=SCALE,
                        )
                        nc.scalar.activation(
                            out=et[:, 1, :], in_=simt[:, 1, :],
                            func=mybir.ActivationFunctionType.Exp, scale=SCALE,
                        )
                        nc.vector.tensor_mul(
                            out=et[:, 1, 0:W], in0=et[:, 1, 0:W], in1=tri_sb[:]
                        )
                    elif g == ns:
                        nc.scalar.activation(
                            out=et[:, 0, 0:W], in_=simt[:, 0, 0:W],
                            func=mybir.ActivationFunctionType.Exp, scale=SCALE,
                        )
                        nc.vector.tensor_mul(
                            out=et[:, 0, 0:W], in0=et[:, 0, 0:W], in1=tri_sb[:]
                        )
                    else:
                        nc.scalar.activation(
                            out=et[:, :, :], in_=simt[:, :, :],
                            func=mybir.ActivationFunctionType.Exp, scale=SCALE,
                        )
                        for sl in range(2):
                            nc.vector.tensor_mul(
                                out=et[:, sl, 0:W], in0=et[:, sl, 0:W], in1=tri_sb[:]
                            )
                    for sl, c in enumerate(blocks):
                        e_tiles[c] = (et, sl)
                    # windows ready after this group
                    for w in ([0] if g == 0 else ([nw - 1] if g == ns else [2 * g - 1, 2 * g])):
                        do_window(w)
                        e_tiles.pop(w - 1, None)

                nc.sync.dma_start(out=nat(o_d[bh]), in_=stage_sb[:])

    nc.finalize()
    return nc


_built = {}
TRACE = False
LAST_RESULT = None


def _get_nc(bh_per_core=BH_PER_CORE, n=N):
    key = (bh_per_core, n)
    if key not in _built:
        _built[key] = build_nc(bh_per_core, n)
    return _built[key]


_runner = None


def _make_runner():
    """Build the jitted SPMD executable ONCE and reuse it across calls.

    run_bass_kernel_spmd constructs a fresh jax.jit(shard_map(...)) closure
    per invocation, so every warm call re-traces + re-lowers + re-runs
    neuronxcc. Caching the jitted callable turns warm calls into pure
    dispatch + transfer + execute.
    """
    import jax
    import jax.numpy as jnp
    from jax.experimental.shard_map import shard_map
    from jax.sharding import Mesh, NamedSharding, PartitionSpec

    from concourse.bass2jax import (
        _bass_exec_p,
        install_neuronx_cc_hook,
        partition_id_tensor,
    )

    install_neuronx_cc_hook()
    nc = _get_nc()
    assert not (nc.dbg_addr is not None and nc.dbg_callbacks)
    partition_name = nc.partition_id_tensor.name if nc.partition_id_tensor else None

    in_names = []
    out_names = []
    out_avals = []
    zero_shapes = []
    for alloc in nc.m.functions[0].allocations:
        if not isinstance(alloc, mybir.MemoryLocationSet):
            continue
        name = alloc.memorylocations[0].name
        if alloc.kind == "ExternalInput":
            if name != partition_name:
                in_names.append(name)
        elif alloc.kind == "ExternalOutput":
            out_names.append(name)
            shape = tuple(alloc.tensor_shape)
            dtype = mybir.dt.np(alloc.dtype)
            out_avals.append(jax.core.ShapedArray(shape, dtype))
            zero_shapes.append((shape, dtype))
    n_params = len(in_names)
    all_in_names = list(in_names) + out_names
    if partition_name is not None:
        all_in_names.append(partition_name)

    def _body(*args):
        operands = list(args)
        if partition_name is not None:
            operands.append(partition_id_tensor())
        outs = _bass_exec_p.bind(
            *operands,
            out_avals=tuple(out_avals),
            in_names=tuple(all_in_names),
            out_names=tuple(out_names),
            lowering_input_output_aliases=(),
            sim_require_finite=True,
            sim_require_nnan=True,
            nc=nc,
        )
        return tuple(outs)

    devices = jax.devices()[:NCORES]
    assert len(devices) == NCORES
    mesh = Mesh(np.asarray(devices), ("core",))
    nspec = n_params + len(out_names)
    sharded = jax.jit(
        shard_map(
            _body,
            mesh=mesh,
            in_specs=(PartitionSpec("core"),) * nspec,
            out_specs=(PartitionSpec("core"),) * len(out_names),
            check_rep=False,
        ),
        donate_argnums=tuple(range(n_params, nspec)),
        keep_unused=True,
    )

    out_sharding = NamedSharding(mesh, PartitionSpec("core"))
    zeros_fns = [
        jax.jit(
            (lambda sh, dt: (lambda: jnp.zeros((NCORES * sh[0], *sh[1:]), dt)))(
                sh, dt
            ),
            out_shardings=out_sharding,
        )
        for sh, dt in zero_shapes
    ]

    # global (concat-over-cores) constant operands: device_put ONCE so warm
    # calls don't re-transfer them
    consts = host_consts(N)
    if nc.dbg_addr is not None:
        consts[nc.dbg_addr.name] = np.zeros((1, 2), np.uint32)
    const_global = {
        name: jax.device_put(
            np.ascontiguousarray(np.tile(arr, (NCORES,) + (1,) * (arr.ndim - 1))),
            out_sharding,
        )
        for name, arr in consts.items()
    }

    def run(qf, kf, vf):
        per_name = {"q": qf, "k": kf, "v": vf, **const_global}
        args = [per_name[name] for name in in_names]
        zeros = [zf() for zf in zeros_fns]
        outs = sharded(*args, *zeros)
        return {name: outs[i] for i, name in enumerate(out_names)}

    return run


def kernel(q, k, v):
    assert q.shape == (B, H, N, D)
    qf = np.ascontiguousarray(q, dtype=np.float32).reshape(BH, N, D)
    kf = np.ascontiguousarray(k, dtype=np.float32).reshape(BH, N, D)
    vf = np.ascontiguousarray(v, dtype=np.float32).reshape(BH, N, D)
    global _runner
    if _runner is None:
        _runner = _make_runner()
    outs = _runner(qf, kf, vf)
    out = np.asarray(outs["out"])
    return out.reshape(B, H, N, D).astype(np.float32, copy=False)



# revision 8
# speedup vs baseline: 5.2936x; 2.4256x over previous
"""Local (windowed) attention with RoPE for Trainium2, SPMD over 8 NeuronCores.

Reference semantics (nn_LocalAttention): B,H,N,D = 4,16,4096,64, window=128,
look_backward=1, look_forward=0, pad_value=-1 (pad applies to k/v VALUES and
to the position ids; padded keys end up unmasked all -1.0 vectors).

Sharding: merged (B*H)=64 leading dim split across 8 cores, 8 slices each.
Everything else runs per-core with no collectives.
"""

import numpy as np
import ml_dtypes

import concourse.bass as bass
import concourse.bacc as bacc
import concourse.mybir as mybir
import concourse.tile as tile
from concourse.bass_utils import run_bass_kernel_spmd

F32 = mybir.dt.float32
BF16 = mybir.dt.bfloat16
NP_BF16 = ml_dtypes.bfloat16

B, H, N, D = 4, 16, 4096, 64
W = 128                    # window size
NCORES = 8
BH = B * H
BH_PER_CORE = BH // NCORES
SCALE = float(D) ** -0.5
HD = D // 2


def rope_tables(n):
    """cos/sin tables matching the reference's fp32 computation.

    sinm folds the rotate_half sign: q'[d] = q[d]*cos[d] + q[(d+32)%64]*sinm[d].
    """
    inv_freq = 1.0 / (10000.0 ** (np.arange(0, D, 2, dtype=np.float32) / np.float32(D)))
    t = np.arange(n, dtype=np.float32)
    half = t[:, None] * inv_freq[None, :]
    freqs = np.concatenate([half, half], axis=-1)  # [n, D]
    cos = np.cos(freqs).astype(np.float32)
    sin = np.sin(freqs).astype(np.float32)
    sinm = np.concatenate([-sin[:, :HD], sin[:, HD:]], axis=-1)
    return cos, sinm


def host_consts(n):
    cos, sinm = rope_tables(n)
    # tri[j, i] = 1 where key j <= query i (window-local causal keep-mask)
    j = np.arange(W)[:, None]
    i = np.arange(W)[None, :]
    tri = (j <= i).astype(NP_BF16)
    ident = np.eye(D + 1, dtype=np.float32)
    return {
        "cos_t": cos.astype(NP_BF16),
        "sinm_t": sinm.astype(NP_BF16),
        "tri": tri,
        "id65": ident,
    }


def build_nc(bh_per_core=BH_PER_CORE, n=N):
    nw = n // W
    assert nw % 2 == 0
    ns = nw // 2  # transpose slabs (2 windows each)

    nc = bacc.Bacc(None, target_bir_lowering=False)
    q_d = nc.dram_tensor("q", [bh_per_core, n, D], BF16, kind="ExternalInput")
    k_d = nc.dram_tensor("k", [bh_per_core, n, D], BF16, kind="ExternalInput")
    v_d = nc.dram_tensor("v", [bh_per_core, n, D], BF16, kind="ExternalInput")
    cos_d = nc.dram_tensor("cos_t", [n, D], BF16, kind="ExternalInput")
    sinm_d = nc.dram_tensor("sinm_t", [n, D], BF16, kind="ExternalInput")
    tri_d = nc.dram_tensor("tri", [W, W], BF16, kind="ExternalInput")
    id_d = nc.dram_tensor("id65", [D + 1, D + 1], F32, kind="ExternalInput")
    o_d = nc.dram_tensor("out", [bh_per_core, n, D], BF16, kind="ExternalOutput")

    def nat(ap):  # DRAM [n, D] -> [t, w, d] token-in-window on partitions
        return ap.rearrange("(w t) d -> t w d", t=W)

    with tile.TileContext(nc) as tc:
        with (
            tc.tile_pool(name="const", bufs=1) as constp,
            tc.tile_pool(name="io", bufs=2) as iop,
            tc.tile_pool(name="rope", bufs=2) as ropep,
            tc.tile_pool(name="stk", bufs=2) as stkp,
            tc.tile_pool(name="esb", bufs=4) as ep,
            tc.tile_pool(name="otsb", bufs=6) as otp,
            tc.tile_pool(name="rsb", bufs=3) as rp,
            tc.tile_pool(name="stage", bufs=2) as stagep,
            tc.tile_pool(name="psim", bufs=2, space="PSUM") as psimp,
            tc.tile_pool(name="pS", bufs=4, space="PSUM") as pSp,
            tc.tile_pool(name="pO", bufs=2, space="PSUM") as pOp,
        ):
            cos_sb = constp.tile([W, nw, D], BF16, tag="cos")
            nc.sync.dma_start(out=cos_sb, in_=nat(cos_d))
            sinm_sb = constp.tile([W, nw, D], BF16, tag="sinm")
            nc.sync.dma_start(out=sinm_sb, in_=nat(sinm_d))
            tri_sb = constp.tile([W, W], BF16, tag="tri")
            nc.sync.dma_start(out=tri_sb, in_=tri_d[:])
            id_sb = constp.tile([D + 1, D + 1], F32, tag="id65")
            nc.sync.dma_start(out=id_sb, in_=id_d[:])
            kpadT = constp.tile([D, W], BF16, tag="kpadT")
            nc.vector.memset(kpadT[:], -1.0)
            vpad = constp.tile([W, D + 1], BF16, tag="vpad")
            nc.vector.memset(vpad[:], -1.0)
            nc.vector.memset(vpad[:, D : D + 1], 1.0)

            for bh in range(bh_per_core):
                qn = iop.tile([W, nw, D], BF16, tag="qn")
                nc.sync.dma_start(out=qn[:], in_=nat(q_d[bh]))
                kn = iop.tile([W, nw, D], BF16, tag="kn")
                nc.sync.dma_start(out=kn[:], in_=nat(k_d[bh]))
                # v lands directly in its ones-column layout (denominator row)
                vb = ropep.tile([W, nw, D + 1], BF16, tag="vb")
                if bh < 2:  # ones column persists per pool slot
                    nc.vector.memset(vb[:, :, D : D + 1], 1.0)
                nc.sync.dma_start(out=vb[:, :, 0:D], in_=nat(v_d[bh]))

                # ---- RoPE (bf16, natural layout) ----
                # Output tiles are [W, nw, 2D] with d-columns D:2D zero -- the
                # XBAR transpose then puts every window's d-major tile at
                # partitions 0:64 (uniform matmul base partition).
                def rope(xb, tag):
                    xr = ropep.tile([W, nw, D], BF16, tag=tag + "r")
                    nc.vector.tensor_mul(
                        out=xr[:, :, 0:HD], in0=xb[:, :, HD:D], in1=sinm_sb[:, :, 0:HD]
                    )
                    nc.vector.tensor_mul(
                        out=xr[:, :, HD:D], in0=xb[:, :, 0:HD], in1=sinm_sb[:, :, HD:D]
                    )
                    xp = ropep.tile([W, nw, 2 * D], BF16, tag=tag + "p")
                    if bh < 2:  # zero the pad lanes once per pool slot
                        nc.vector.memset(xp[:, :, D : 2 * D], 0.0)
                    nc.vector.tensor_mul(out=xp[:, :, 0:D], in0=xb[:], in1=cos_sb[:])
                    nc.vector.tensor_add(
                        out=xp[:, :, 0:D], in0=xp[:, :, 0:D], in1=xr[:]
                    )
                    return xp

                qp = rope(qn, "q")
                kp = rope(kn, "k")

                # ---- d-major via XBAR dma transpose ----
                # stq[p, w, t]: p<64 -> d of window w; p>=64 -> zero pad
                stq = stkp.tile([W, nw, W], BF16, tag="stq")
                nc.sync.dma_start(
                    out=stq[:], in_=qp.rearrange("t w d -> t (w d)"), transpose=True
                )
                stk = stkp.tile([W, nw, W], BF16, tag="stk")
                nc.sync.dma_start(
                    out=stk[:], in_=kp.rearrange("t w d -> t (w d)"), transpose=True
                )

                def qT(w):  # [64, 128] moving operand for queries of window w
                    return stq[0:D, w, :]

                def kT(w):  # [64, 128] stationary operand for keys of window w
                    return stk[0:D, w, :]

                # groups of key blocks: g=0 -> (pad, 0); 1..ns-1 -> (2g-1, 2g);
                # g=ns -> (nw-1,)
                e_tiles = {}  # c -> (E tile, slot)
                o_quads = {}
                stage_sb = stagep.tile([W, nw, D], BF16, tag="stage")

                def do_window(w):
                    # out^T (and denom) for window w: accumulate both key
                    # blocks' PV into one PSUM tile, evacuate, transpose.
                    et0, sl0 = e_tiles[w - 1]
                    et1, sl1 = e_tiles[w]
                    pw = pSp.tile([D + 1, W], F32, tag="s", name="pw")
                    if w == 0:
                        nc.tensor.matmul(
                            pw[:], vpad[:], et0[:, sl0, 0:W], start=True, stop=False
                        )
                    else:
                        nc.tensor.matmul(
                            pw[:], vb[:, w - 1, :], et0[:, sl0, W : 2 * W],
                            start=True, stop=False,
                        )
                    nc.tensor.matmul(
                        pw[:], vb[:, w, :], et1[:, sl1, 0:W], start=False, stop=True
                    )
                    ot = otp.tile([D + 1, W], F32, tag="ot")
                    if w % 4 == 2:  # shed some PSUM-evac load from DVE to ACT
                        nc.scalar.copy(out=ot[:], in_=pw[:])
                    else:
                        nc.vector.tensor_copy(out=ot[:], in_=pw[:])
                    qi = w // 4
                    if qi not in o_quads:
                        o_quads[qi] = pOp.tile([W, 4, D + 1], F32, tag="oq", name="oq")
                    oq = o_quads[qi]
                    sl = w % 4
                    nc.tensor.transpose(oq[:, sl, :], ot[:], id_sb[:])
                    if sl == 3 or w == nw - 1:
                        nsl = sl + 1
                        r = rp.tile([W, 4], F32, tag="r")
                        nc.vector.reciprocal(
                            out=r[:, 0:nsl], in_=oq[:, 0:nsl, D : D + 1]
                        )
                        for j in range(nsl):
                            ww = qi * 4 + j
                            nc.scalar.activation(
                                out=stage_sb[:, ww, :],
                                in_=oq[:, j, 0:D],
                                func=mybir.ActivationFunctionType.Copy,
                                scale=r[:, j : j + 1],
                            )

                for g in range(ns + 1):
                    blocks = (
                        [-1, 0] if g == 0 else ([nw - 1] if g == ns else [2 * g - 1, 2 * g])
                    )
                    simt = psimp.tile([W, 2, 2 * W], F32, tag="sim")
                    et = ep.tile([W, 2, 2 * W], BF16, tag="e")
                    for sl, c in enumerate(blocks):
                        last = c == nw - 1
                        if c == -1:
                            nc.tensor.matmul(
                                simt[:, sl, 0:W], kpadT[:], qT(0), start=True, stop=True
                            )
                        else:
                            nc.tensor.matmul(
                                simt[:, sl, 0:W], kT(c), qT(c), start=True, stop=True
                            )
                            if not last:
                                nc.tensor.matmul(
                                    simt[:, sl, W : 2 * W],
                                    kT(c),
                                    qT(c + 1),
                                    start=True,
                                    stop=True,
                                )
                    # exp (scale folded); masked entries fixed up after
                    if g == 0:
                        nc.scalar.activation(
                            out=et[:, 0, 0:W], in_=simt[:, 0, 0:W],
                            func=mybir.ActivationFunctionType.Exp, scale=SCALE,
                        )
                        nc.scalar.activation(
                            out=et[:, 1, :], in_=simt[:, 1, :],
                            func=mybir.ActivationFunctionType.Exp, scale=SCALE,
                        )
                        nc.vector.tensor_mul(
                            out=et[:, 1, 0:W], in0=et[:, 1, 0:W], in1=tri_sb[:]
                        )
                    elif g == ns:
                        nc.scalar.activation(
                            out=et[:, 0, 0:W], in_=simt[:, 0, 0:W],
                            func=mybir.ActivationFunctionType.Exp, scale=SCALE,
                        )
                        nc.vector.tensor_mul(
                            out=et[:, 0, 0:W], in0=et[:, 0, 0:W], in1=tri_sb[:]
                        )
                    else:
                        nc.scalar.activation(
                            out=et[:, :, :], in_=simt[:, :, :],
                            func=mybir.ActivationFunctionType.Exp, scale=SCALE,
                        )
                        for sl in range(2):
                            nc.vector.tensor_mul(
                                out=et[:, sl, 0:W], in0=et[:, sl, 0:W], in1=tri_sb[:]
                            )
                    for sl, c in enumerate(blocks):
                        e_tiles[c] = (et, sl)
                    # windows ready after this group
                    for w in ([0] if g == 0 else ([nw - 1] if g == ns else [2 * g - 1, 2 * g])):
                        do_window(w)
                        e_tiles.pop(w - 1, None)

                nc.sync.dma_start(out=nat(o_d[bh]), in_=stage_sb[:])

    nc.finalize()
    return nc


_built = {}
TRACE = False
LAST_RESULT = None


def _get_nc(bh_per_core=BH_PER_CORE, n=N):
    key = (bh_per_core, n)
    if key not in _built:
        _built[key] = build_nc(bh_per_core, n)
    return _built[key]


_runner = None


def _make_runner():
    """Build the jitted SPMD executable ONCE and reuse it across calls.

    run_bass_kernel_spmd constructs a fresh jax.jit(shard_map(...)) closure
    per invocation, so every warm call re-traces + re-lowers + re-runs
    neuronxcc. Caching the jitted callable turns warm calls into pure
    dispatch + transfer + execute.
    """
    import jax
    import jax.numpy as jnp
    from jax.experimental.shard_map import shard_map
    from jax.sharding import Mesh, NamedSharding, PartitionSpec

    from concourse.bass2jax import (
        _bass_exec_p,
        install_neuronx_cc_hook,
        partition_id_tensor,
    )

    install_neuronx_cc_hook()
    nc = _get_nc()
    assert not (nc.dbg_addr is not None and nc.dbg_callbacks)
    partition_name = nc.partition_id_tensor.name if nc.partition_id_tensor else None

    in_names = []
    out_names = []
    out_avals = []
    zero_shapes = []
    for alloc in nc.m.functions[0].allocations:
        if not isinstance(alloc, mybir.MemoryLocationSet):
            continue
        name = alloc.memorylocations[0].name
        if alloc.kind == "ExternalInput":
            if name != partition_name:
                in_names.append(name)
        elif alloc.kind == "ExternalOutput":
            out_names.append(name)
            shape = tuple(alloc.tensor_shape)
            dtype = mybir.dt.np(alloc.dtype)
            out_avals.append(jax.core.ShapedArray(shape, dtype))
            zero_shapes.append((shape, dtype))
    n_params = len(in_names)
    all_in_names = list(in_names) + out_names
    if partition_name is not None:
        all_in_names.append(partition_name)

    def _body(*args):
        operands = list(args)
        if partition_name is not None:
            operands.append(partition_id_tensor())
        outs = _bass_exec_p.bind(
            *operands,
            out_avals=tuple(out_avals),
            in_names=tuple(all_in_names),
            out_names=tuple(out_names),
            lowering_input_output_aliases=(),
            sim_require_finite=True,
            sim_require_nnan=True,
            nc=nc,
        )
        return tuple(outs)

    devices = jax.devices()[:NCORES]
    assert len(devices) == NCORES
    mesh = Mesh(np.asarray(devices), ("core",))
    nspec = n_params + len(out_names)
    sharded = jax.jit(
        shard_map(
            _body,
            mesh=mesh,
            in_specs=(PartitionSpec("core"),) * nspec,
            out_specs=(PartitionSpec("core"),) * len(out_names),
            check_rep=False,
        ),
        donate_argnums=tuple(range(n_params, nspec)),
        keep_unused=True,
    )

    out_sharding = NamedSharding(mesh, PartitionSpec("core"))
    zeros_fns = [
        jax.jit(
            (lambda sh, dt: (lambda: jnp.zeros((NCORES * sh[0], *sh[1:]), dt)))(
                sh, dt
            ),
            out_shardings=out_sharding,
        )
        for sh, dt in zero_shapes
    ]

    # global (concat-over-cores) constant operands: device_put ONCE so warm
    # calls don't re-transfer them
    consts = host_consts(N)
    if nc.dbg_addr is not None:
        consts[nc.dbg_addr.name] = np.zeros((1, 2), np.uint32)
    const_global = {
        name: jax.device_put(
            np.ascontiguousarray(np.tile(arr, (NCORES,) + (1,) * (arr.ndim - 1))),
            out_sharding,
        )
        for name, arr in consts.items()
    }

    def run(qf, kf, vf):
        per_name = {"q": qf, "k": kf, "v": vf, **const_global}
        args = [per_name[name] for name in in_names]
        zeros = [zf() for zf in zeros_fns]
        outs = sharded(*args, *zeros)
        return {name: outs[i] for i, name in enumerate(out_names)}

    return run


def kernel(q, k, v):
    assert q.shape == (B, H, N, D)
    qf = np.asarray(q).reshape(BH, N, D).astype(NP_BF16)
    kf = np.asarray(k).reshape(BH, N, D).astype(NP_BF16)
    vf = np.asarray(v).reshape(BH, N, D).astype(NP_BF16)
    global _runner
    if _runner is None:
        _runner = _make_runner()
    outs = _runner(qf, kf, vf)
    out = np.asarray(outs["out"])
    return out.reshape(B, H, N, D).astype(np.float32)

